# revision 11
# baseline (speedup 1.0000x reference)
"""DWT-attention Trainium2 kernel.

Math (per batch b, all on device):
  out = ( iDWT3( W ⊙ (DWT3(x)Wq^T) ⊙ (DWT3(x)Wv^T) ) ) Wout^T + b_out
using that the Haar DWT along L commutes with channel projections, so the
DWT is applied ONCE to x (not to q and v separately), and all 1/sqrt(2)
band scales + the per-(head,channel) band weights are folded into
per-channel scalars applied on the PSUM->SBUF copy.

Layout: everything transposed — x^T [D, L] so L is the free dim (DWT is
free-dim strided adds on DVE) and channels are partitions (band weights
become per-partition scalars). Sharding: batch B=8, one batch per core.
Matmuls run in float32r (fast-FP32 PE mode, ~1e-4 rel err).
"""
import sys
sys.path.insert(0, "/opt/trn_rl_repo")
import numpy as np

B, L, D, H, NMODE, Dh = 8, 4096, 1024, 16, 3, 64
LC = 1024                  # L-chunk
NCHUNK = L // LC
KT = D // 128              # k tiles (contraction)
MT = D // 128              # m tiles (output channels)
INVSQRT2 = 0.7071067811865476

_CACHE = {}


def _build():
    import concourse.bacc as bacc
    import concourse.mybir as mybir
    import concourse.tile as tile

    f32, f32r = mybir.dt.float32, mybir.dt.float32r
    ident = mybir.ActivationFunctionType.Identity
    add_op, mult_op = mybir.AluOpType.add, mybir.AluOpType.mult
    C3 = INVSQRT2 ** 3

    nc = bacc.Bacc("TRN2", target_bir_lowering=False, debug=False)
    xt = nc.dram_tensor("xt", [D, L], f32, kind="ExternalInput")
    wqt = nc.dram_tensor("wqt", [D, D], f32r, kind="ExternalInput")
    wvt = nc.dram_tensor("wvt", [D, D], f32r, kind="ExternalInput")
    wot = nc.dram_tensor("wot", [D, D], f32r, kind="ExternalInput")
    scal = nc.dram_tensor("scal", [128, 48], f32, kind="ExternalInput")
    outt = nc.dram_tensor("outt", [D, L], f32, kind="ExternalOutput")

    with tile.TileContext(nc) as tc:
        with tc.tile_pool(name="wq", bufs=KT * 4) as wq_pool, \
             tc.tile_pool(name="wv", bufs=KT * 4) as wv_pool, \
             tc.tile_pool(name="wo", bufs=KT) as wo_pool, \
             tc.tile_pool(name="const", bufs=1) as const_pool, \
             tc.tile_pool(name="x", bufs=2) as x_pool, \
             tc.tile_pool(name="u", bufs=8) as u_pool, \
             tc.tile_pool(name="ta", bufs=2) as ta_pool, \
             tc.tile_pool(name="cq", bufs=2) as cq_pool, \
             tc.tile_pool(name="prod", bufs=2) as prod_pool, \
             tc.tile_pool(name="ti", bufs=2) as ti_pool, \
             tc.tile_pool(name="y", bufs=8) as y_pool, \
             tc.tile_pool(name="o", bufs=2) as o_pool, \
             tc.tile_pool(name="psum", bufs=8, space="PSUM") as psum_pool:

            scal_sb = const_pool.tile([128, 48], f32)
            nc.sync.dma_start(scal_sb[:], scal.ap())

            def sap(j, m):          # per-partition scalar column
                return scal_sb[:, j * 8 + m: j * 8 + m + 1]

            def dwt_tile(k, csl):
                """Load x^T tile and produce its unscaled Haar-3 bands."""
                xt_t = x_pool.tile([128, LC], f32, tag="x")
                nc.sync.dma_start(xt_t[:], xt.ap()[k * 128:(k + 1) * 128, csl])
                ut = u_pool.tile([128, LC], f32r, tag="u")
                e, o = xt_t[:, 0:LC:2], xt_t[:, 1:LC:2]
                nc.vector.tensor_sub(ut[:, 512:1024], e, o)          # D1
                t1 = ta_pool.tile([128, 512], f32, tag="t1")
                nc.gpsimd.tensor_add(t1[:], e, o)                    # A1
                e2, o2 = t1[:, 0:512:2], t1[:, 1:512:2]
                nc.vector.tensor_sub(ut[:, 256:512], e2, o2)         # D2
                t2 = ta_pool.tile([128, 256], f32, tag="t2")
                nc.gpsimd.tensor_add(t2[:], e2, o2)                  # A2
                e3, o3 = t2[:, 0:256:2], t2[:, 1:256:2]
                nc.vector.tensor_sub(ut[:, 128:256], e3, o3)         # D3
                nc.vector.tensor_add(ut[:, 0:128], e3, o3)           # A3
                return ut

            # startup: weights in 256-col m-groups, streamed in the order
            # the PE consumes them (group 0 of wq+wv, then x chunk 0, then
            # the remaining groups); wo (stage-D only) last.
            GW = 4  # m-groups per weight k-tile
            wq_t = [[None] * GW for _ in range(KT)]
            wv_t = [[None] * GW for _ in range(KT)]
            wo_t, u_first = [], []

            def load_wgrp(pool, tag, src, k, g):
                sl = slice(k * 128, (k + 1) * 128)
                gsl = slice(g * 256, (g + 1) * 256)
                t = pool.tile([128, 256], f32r, tag=tag)
                nc.sync.dma_start(t[:], src.ap()[sl, gsl])
                return t

            for k in range(KT):
                wq_t[k][0] = load_wgrp(wq_pool, "wq", wqt, k, 0)
                wv_t[k][0] = load_wgrp(wv_pool, "wv", wvt, k, 0)
            for k in range(KT):
                u_first.append(dwt_tile(k, slice(0, LC)))
            for g in range(1, GW):
                for k in range(KT):
                    wq_t[k][g] = load_wgrp(wq_pool, "wq", wqt, k, g)
                    wv_t[k][g] = load_wgrp(wv_pool, "wv", wvt, k, g)
            for k in range(KT):
                sl = slice(k * 128, (k + 1) * 128)
                t = wo_pool.tile([128, D], f32r, tag="wo")
                nc.sync.dma_start(t[:], wot.ap()[sl, :])
                wo_t.append(t)

            def wqap(k, m):
                return wq_t[k][m // 2][:, (m % 2) * 128:(m % 2) * 128 + 128]

            def wvap(k, m):
                return wv_t[k][m // 2][:, (m % 2) * 128:(m % 2) * 128 + 128]

            for c in range(NCHUNK):
                csl = slice(c * LC, (c + 1) * LC)
                # ---- stage A: unscaled Haar-3 DWT of x^T chunk ----
                if c == 0:
                    u_c = u_first
                else:
                    u_c = [dwt_tile(k, csl) for k in range(KT)]

                # ---- stages B (project+weight+product) and C (iDWT) ----
                y_c = []
                for m in range(MT):
                    msl = slice(m * 128, (m + 1) * 128)
                    prod = prod_pool.tile([128, LC], f32, tag="prod")
                    # paired n-halves per stationary block (one LDW, 2 MMs)
                    psq0 = psum_pool.tile([128, 512], f32, tag="ps")
                    psq1 = psum_pool.tile([128, 512], f32, tag="ps")
                    for k in range(KT):
                        nc.tensor.matmul(psq0[:], wqap(k, m), u_c[k][:, 0:512],
                                         start=(k == 0), stop=(k == KT - 1))
                        nc.tensor.matmul(psq1[:], wqap(k, m), u_c[k][:, 512:1024],
                                         start=(k == 0), stop=(k == KT - 1))
                    psv0 = psum_pool.tile([128, 512], f32, tag="ps")
                    psv1 = psum_pool.tile([128, 512], f32, tag="ps")
                    for k in range(KT):
                        nc.tensor.matmul(psv0[:], wvap(k, m), u_c[k][:, 0:512],
                                         start=(k == 0), stop=(k == KT - 1))
                        nc.tensor.matmul(psv1[:], wvap(k, m), u_c[k][:, 512:1024],
                                         start=(k == 0), stop=(k == KT - 1))
                    cq = cq_pool.tile([128, 1024], f32, tag="cq")
                    # bands [A3 0:128 | D3 128:256 | D2 256:512 | D1 512:1024]
                    nc.scalar.activation(cq[:, 0:128], psq0[:, 0:128], ident,
                                         bias=sap(3, m), scale=sap(0, m))
                    nc.scalar.mul(cq[:, 128:256], psq0[:, 128:256], sap(1, m))
                    nc.scalar.mul(cq[:, 256:512], psq0[:, 256:512], sap(2, m))
                    nc.scalar.mul(cq[:, 512:1024], psq1[:], C3)
                    nc.vector.scalar_tensor_tensor(
                        prod[:, 0:128], psv0[:, 0:128], sap(4, m), cq[:, 0:128],
                        op0=add_op, op1=mult_op)
                    nc.vector.tensor_mul(prod[:, 128:512], cq[:, 128:512],
                                         psv0[:, 128:512])
                    nc.vector.tensor_mul(prod[:, 512:1024], cq[:, 512:1024], psv1[:])
                    # iDWT (unscaled butterflies, interleaved writes)
                    it2 = ti_pool.tile([128, 256], f32, tag="it2")
                    nc.vector.tensor_add(it2[:, 0:256:2], prod[:, 0:128], prod[:, 128:256])
                    nc.vector.tensor_sub(it2[:, 1:256:2], prod[:, 0:128], prod[:, 128:256])
                    it1 = ti_pool.tile([128, 512], f32, tag="it1")
                    nc.vector.tensor_add(it1[:, 0:512:2], it2[:], prod[:, 256:512])
                    nc.vector.tensor_sub(it1[:, 1:512:2], it2[:], prod[:, 256:512])
                    yt = y_pool.tile([128, LC], f32r, tag="y")
                    nc.vector.tensor_add(yt[:, 0:LC:2], it1[:], prod[:, 512:1024])
                    nc.vector.tensor_sub(yt[:, 1:LC:2], it1[:], prod[:, 512:1024])
                    y_c.append(yt)

                # ---- stage D: output projection (paired n-halves) ----
                for m in range(MT):
                    msl = slice(m * 128, (m + 1) * 128)
                    pso0 = psum_pool.tile([128, 512], f32, tag="ps")
                    pso1 = psum_pool.tile([128, 512], f32, tag="ps")
                    for k in range(KT):
                        nc.tensor.matmul(pso0[:], wo_t[k][:, msl], y_c[k][:, 0:512],
                                         start=(k == 0), stop=(k == KT - 1))
                        nc.tensor.matmul(pso1[:], wo_t[k][:, msl], y_c[k][:, 512:1024],
                                         start=(k == 0), stop=(k == KT - 1))
                    ot = o_pool.tile([128, 1024], f32, tag="o")
                    nc.scalar.activation(ot[:, 0:512], pso0[:], ident,
                                         bias=sap(5, m), scale=1.0)
                    nc.scalar.activation(ot[:, 512:1024], pso1[:], ident,
                                         bias=sap(5, m), scale=1.0)
                    nc.sync.dma_start(outt.ap()[msl, csl], ot[:])
    nc.compile()
    return nc


def _prep(W_qkv, b_qkv, W_out, b_out, weight_q, weight_v):
    c = INVSQRT2
    WqT = np.ascontiguousarray(W_qkv[0:D, :].T, dtype=np.float32)
    WvT = np.ascontiguousarray(W_qkv[2 * D:3 * D, :].T, dtype=np.float32)
    WoT = np.ascontiguousarray(W_out.T, dtype=np.float32)
    wb = np.empty((3, D), np.float32)
    for j in range(3):
        wb[j] = (np.asarray(weight_q)[:, j, :] * np.asarray(weight_v)[:, j, :]).reshape(D)
    wb0 = wb[0] * c ** 9
    wb1 = wb[1] * c ** 9
    wb2 = wb[2] * c ** 6
    bq = np.asarray(b_qkv)[0:D].astype(np.float32)
    bv = np.asarray(b_qkv)[2 * D:3 * D].astype(np.float32)
    bqw8 = wb0 * 8.0 * bq
    bv8 = 8.0 * bv
    bout = np.asarray(b_out).astype(np.float32)
    scal = np.empty((128, 48), np.float32)
    for j, vec in enumerate((wb0, wb1, wb2, bqw8, bv8, bout)):
        scal[:, j * 8:(j + 1) * 8] = vec.reshape(8, 128).T
    return WqT, WvT, WoT, scal


def kernel(query, W_qkv, b_qkv, W_out, b_out, weight_q, weight_v, _trace=False):
    from concourse.bass_utils import run_bass_kernel_spmd

    if "nc" not in _CACHE:
        _CACHE["nc"] = _build()
    nc = _CACHE["nc"]

    WqT, WvT, WoT, scal = _prep(W_qkv, b_qkv, W_out, b_out, weight_q, weight_v)
    query = np.asarray(query, dtype=np.float32)
    in_maps = []
    for b in range(B):
        in_maps.append({
            "xt": np.ascontiguousarray(query[b].T),
            "wqt": WqT, "wvt": WvT, "wot": WoT, "scal": scal,
        })
    res = run_bass_kernel_spmd(nc, in_maps, list(range(B)), trace=_trace)
    out = np.empty((B, L, D), np.float32)
    for b in range(B):
        out[b] = res.results[b]["outt"].T
    if _trace:
        _CACHE["last_results"] = res
    return out


# revision 12
# speedup vs baseline: 1.0087x; 1.0087x over previous
"""DWT-attention Trainium2 kernel.

Math (per batch b, all on device):
  out = ( iDWT3( W ⊙ (DWT3(x)Wq^T) ⊙ (DWT3(x)Wv^T) ) ) Wout^T + b_out
using that the Haar DWT along L commutes with channel projections, so the
DWT is applied ONCE to x (not to q and v separately), and all 1/sqrt(2)
band scales + the per-(head,channel) band weights are folded into
per-channel scalars applied on the PSUM->SBUF copy.

Layout: everything transposed — x^T [D, L] so L is the free dim (DWT is
free-dim strided adds on DVE) and channels are partitions (band weights
become per-partition scalars). Sharding: batch B=8, one batch per core.
Matmuls run in float32r (fast-FP32 PE mode, ~1e-4 rel err).
"""
import sys
sys.path.insert(0, "/opt/trn_rl_repo")
import numpy as np

B, L, D, H, NMODE, Dh = 8, 4096, 1024, 16, 3, 64
LC = 1024                  # L-chunk
NCHUNK = L // LC
KT = D // 128              # k tiles (contraction)
MT = D // 128              # m tiles (output channels)
INVSQRT2 = 0.7071067811865476

_CACHE = {}


def _build():
    import concourse.bacc as bacc
    import concourse.mybir as mybir
    import concourse.tile as tile

    f32, f32r = mybir.dt.float32, mybir.dt.float32r
    ident = mybir.ActivationFunctionType.Identity
    add_op, mult_op = mybir.AluOpType.add, mybir.AluOpType.mult
    C3 = INVSQRT2 ** 3

    nc = bacc.Bacc("TRN2", target_bir_lowering=False, debug=False)
    xt = nc.dram_tensor("xt", [D, L], f32, kind="ExternalInput")
    wqt = nc.dram_tensor("wqt", [D, D], f32r, kind="ExternalInput")
    wvt = nc.dram_tensor("wvt", [D, D], f32r, kind="ExternalInput")
    wot = nc.dram_tensor("wot", [D, D], f32r, kind="ExternalInput")
    scal = nc.dram_tensor("scal", [128, 48], f32, kind="ExternalInput")
    outt = nc.dram_tensor("outt", [D, L], f32, kind="ExternalOutput")

    with tile.TileContext(nc) as tc:
        with tc.tile_pool(name="wq", bufs=KT) as wq_pool, \
             tc.tile_pool(name="wv", bufs=KT) as wv_pool, \
             tc.tile_pool(name="wo", bufs=KT) as wo_pool, \
             tc.tile_pool(name="const", bufs=1) as const_pool, \
             tc.tile_pool(name="x", bufs=2) as x_pool, \
             tc.tile_pool(name="u", bufs=8) as u_pool, \
             tc.tile_pool(name="ta", bufs=2) as ta_pool, \
             tc.tile_pool(name="cq", bufs=2) as cq_pool, \
             tc.tile_pool(name="prod", bufs=2) as prod_pool, \
             tc.tile_pool(name="ti", bufs=2) as ti_pool, \
             tc.tile_pool(name="y", bufs=8) as y_pool, \
             tc.tile_pool(name="o", bufs=2) as o_pool, \
             tc.tile_pool(name="psum", bufs=8, space="PSUM") as psum_pool:

            scal_sb = const_pool.tile([128, 48], f32)
            nc.sync.dma_start(scal_sb[:], scal.ap())

            def sap(j, m):          # per-partition scalar column
                return scal_sb[:, j * 8 + m: j * 8 + m + 1]

            def dwt_tile(k, csl):
                """Load x^T tile and produce its unscaled Haar-3 bands."""
                xt_t = x_pool.tile([128, LC], f32, tag="x")
                nc.sync.dma_start(xt_t[:], xt.ap()[k * 128:(k + 1) * 128, csl])
                ut = u_pool.tile([128, LC], f32r, tag="u")
                e, o = xt_t[:, 0:LC:2], xt_t[:, 1:LC:2]
                nc.vector.tensor_sub(ut[:, 512:1024], e, o)          # D1
                t1 = ta_pool.tile([128, 512], f32, tag="t1")
                nc.vector.tensor_add(t1[:], e, o)                    # A1
                e2, o2 = t1[:, 0:512:2], t1[:, 1:512:2]
                nc.vector.tensor_sub(ut[:, 256:512], e2, o2)         # D2
                t2 = ta_pool.tile([128, 256], f32, tag="t2")
                nc.vector.tensor_add(t2[:], e2, o2)                  # A2
                e3, o3 = t2[:, 0:256:2], t2[:, 1:256:2]
                nc.vector.tensor_sub(ut[:, 128:256], e3, o3)         # D3
                nc.vector.tensor_add(ut[:, 0:128], e3, o3)           # A3
                return ut

            # startup: x chunk-0 first (feeds DWT), then wq, wv; wo last.
            u_first = [dwt_tile(k, slice(0, LC)) for k in range(KT)]
            wq_t, wv_t, wo_t = [], [], []
            for k in range(KT):
                sl = slice(k * 128, (k + 1) * 128)
                t = wq_pool.tile([128, D], f32r, tag="wq")
                nc.sync.dma_start(t[:], wqt.ap()[sl, :])
                wq_t.append(t)
            for k in range(KT):
                sl = slice(k * 128, (k + 1) * 128)
                t = wv_pool.tile([128, D], f32r, tag="wv")
                nc.sync.dma_start(t[:], wvt.ap()[sl, :])
                wv_t.append(t)
            for k in range(KT):
                sl = slice(k * 128, (k + 1) * 128)
                t = wo_pool.tile([128, D], f32r, tag="wo")
                nc.sync.dma_start(t[:], wot.ap()[sl, :])
                wo_t.append(t)

            def wqap(k, m):
                return wq_t[k][:, m * 128:(m + 1) * 128]

            def wvap(k, m):
                return wv_t[k][:, m * 128:(m + 1) * 128]

            for c in range(NCHUNK):
                csl = slice(c * LC, (c + 1) * LC)
                # ---- stage A: unscaled Haar-3 DWT of x^T chunk ----
                if c == 0:
                    u_c = u_first
                else:
                    u_c = [dwt_tile(k, csl) for k in range(KT)]

                # ---- stages B (project+weight+product) and C (iDWT) ----
                y_c = []
                for m in range(MT):
                    msl = slice(m * 128, (m + 1) * 128)
                    prod = prod_pool.tile([128, LC], f32, tag="prod")
                    # paired n-halves per stationary block (one LDW, 2 MMs)
                    psq0 = psum_pool.tile([128, 512], f32, tag="ps")
                    psq1 = psum_pool.tile([128, 512], f32, tag="ps")
                    for k in range(KT):
                        nc.tensor.matmul(psq0[:], wqap(k, m), u_c[k][:, 0:512],
                                         start=(k == 0), stop=(k == KT - 1))
                        nc.tensor.matmul(psq1[:], wqap(k, m), u_c[k][:, 512:1024],
                                         start=(k == 0), stop=(k == KT - 1))
                    psv0 = psum_pool.tile([128, 512], f32, tag="ps")
                    psv1 = psum_pool.tile([128, 512], f32, tag="ps")
                    for k in range(KT):
                        nc.tensor.matmul(psv0[:], wvap(k, m), u_c[k][:, 0:512],
                                         start=(k == 0), stop=(k == KT - 1))
                        nc.tensor.matmul(psv1[:], wvap(k, m), u_c[k][:, 512:1024],
                                         start=(k == 0), stop=(k == KT - 1))
                    cq = cq_pool.tile([128, 1024], f32, tag="cq")
                    # bands [A3 0:128 | D3 128:256 | D2 256:512 | D1 512:1024]
                    nc.scalar.activation(cq[:, 0:128], psq0[:, 0:128], ident,
                                         bias=sap(3, m), scale=sap(0, m))
                    nc.scalar.mul(cq[:, 128:256], psq0[:, 128:256], sap(1, m))
                    nc.scalar.mul(cq[:, 256:512], psq0[:, 256:512], sap(2, m))
                    nc.scalar.mul(cq[:, 512:1024], psq1[:], C3)
                    nc.vector.scalar_tensor_tensor(
                        prod[:, 0:128], psv0[:, 0:128], sap(4, m), cq[:, 0:128],
                        op0=add_op, op1=mult_op)
                    nc.vector.tensor_mul(prod[:, 128:512], cq[:, 128:512],
                                         psv0[:, 128:512])
                    nc.vector.tensor_mul(prod[:, 512:1024], cq[:, 512:1024], psv1[:])
                    # iDWT (unscaled butterflies, interleaved writes)
                    it2 = ti_pool.tile([128, 256], f32, tag="it2")
                    nc.vector.tensor_add(it2[:, 0:256:2], prod[:, 0:128], prod[:, 128:256])
                    nc.vector.tensor_sub(it2[:, 1:256:2], prod[:, 0:128], prod[:, 128:256])
                    it1 = ti_pool.tile([128, 512], f32, tag="it1")
                    nc.vector.tensor_add(it1[:, 0:512:2], it2[:], prod[:, 256:512])
                    nc.vector.tensor_sub(it1[:, 1:512:2], it2[:], prod[:, 256:512])
                    yt = y_pool.tile([128, LC], f32r, tag="y")
                    nc.vector.tensor_add(yt[:, 0:LC:2], it1[:], prod[:, 512:1024])
                    nc.vector.tensor_sub(yt[:, 1:LC:2], it1[:], prod[:, 512:1024])
                    y_c.append(yt)

                # ---- stage D: output projection (paired n-halves) ----
                for m in range(MT):
                    msl = slice(m * 128, (m + 1) * 128)
                    pso0 = psum_pool.tile([128, 512], f32, tag="ps")
                    pso1 = psum_pool.tile([128, 512], f32, tag="ps")
                    for k in range(KT):
                        nc.tensor.matmul(pso0[:], wo_t[k][:, msl], y_c[k][:, 0:512],
                                         start=(k == 0), stop=(k == KT - 1))
                        nc.tensor.matmul(pso1[:], wo_t[k][:, msl], y_c[k][:, 512:1024],
                                         start=(k == 0), stop=(k == KT - 1))
                    ot = o_pool.tile([128, 1024], f32, tag="o")
                    nc.scalar.activation(ot[:, 0:512], pso0[:], ident,
                                         bias=sap(5, m), scale=1.0)
                    nc.scalar.activation(ot[:, 512:1024], pso1[:], ident,
                                         bias=sap(5, m), scale=1.0)
                    nc.sync.dma_start(outt.ap()[msl, csl], ot[:])
    nc.compile()
    return nc


def _prep(W_qkv, b_qkv, W_out, b_out, weight_q, weight_v):
    c = INVSQRT2
    WqT = np.ascontiguousarray(W_qkv[0:D, :].T, dtype=np.float32)
    WvT = np.ascontiguousarray(W_qkv[2 * D:3 * D, :].T, dtype=np.float32)
    WoT = np.ascontiguousarray(W_out.T, dtype=np.float32)
    wb = np.empty((3, D), np.float32)
    for j in range(3):
        wb[j] = (np.asarray(weight_q)[:, j, :] * np.asarray(weight_v)[:, j, :]).reshape(D)
    wb0 = wb[0] * c ** 9
    wb1 = wb[1] * c ** 9
    wb2 = wb[2] * c ** 6
    bq = np.asarray(b_qkv)[0:D].astype(np.float32)
    bv = np.asarray(b_qkv)[2 * D:3 * D].astype(np.float32)
    bqw8 = wb0 * 8.0 * bq
    bv8 = 8.0 * bv
    bout = np.asarray(b_out).astype(np.float32)
    scal = np.empty((128, 48), np.float32)
    for j, vec in enumerate((wb0, wb1, wb2, bqw8, bv8, bout)):
        scal[:, j * 8:(j + 1) * 8] = vec.reshape(8, 128).T
    return WqT, WvT, WoT, scal


def kernel(query, W_qkv, b_qkv, W_out, b_out, weight_q, weight_v, _trace=False):
    from concourse.bass_utils import run_bass_kernel_spmd

    if "nc" not in _CACHE:
        _CACHE["nc"] = _build()
    nc = _CACHE["nc"]

    WqT, WvT, WoT, scal = _prep(W_qkv, b_qkv, W_out, b_out, weight_q, weight_v)
    query = np.asarray(query, dtype=np.float32)
    in_maps = []
    for b in range(B):
        in_maps.append({
            "xt": np.ascontiguousarray(query[b].T),
            "wqt": WqT, "wvt": WvT, "wot": WoT, "scal": scal,
        })
    res = run_bass_kernel_spmd(nc, in_maps, list(range(B)), trace=_trace)
    out = np.empty((B, L, D), np.float32)
    for b in range(B):
        out[b] = res.results[b]["outt"].T
    if _trace:
        _CACHE["last_results"] = res
    return out


# revision 14
# speedup vs baseline: 1.0218x; 1.0130x over previous
"""DWT-attention Trainium2 kernel.

Math (per batch b, all on device):
  out = ( iDWT3( W ⊙ (DWT3(x)Wq^T) ⊙ (DWT3(x)Wv^T) ) ) Wout^T + b_out
using that the Haar DWT along L commutes with channel projections, so the
DWT is applied ONCE to x (not to q and v separately), and all 1/sqrt(2)
band scales + the per-(head,channel) band weights are folded into
per-channel scalars applied on the PSUM->SBUF copy. b_out is added on the
host (it is additive at the very end).

Layout: everything transposed — x^T [D, L] so L is the free dim (DWT is
free-dim strided adds on DVE) and channels are partitions (band weights
become per-partition scalars). Sharding: batch B=8, one batch per core.
Matmuls run in float32r (fast-FP32 PE mode, ~1e-4 rel err).
"""
import sys
sys.path.insert(0, "/opt/trn_rl_repo")
import numpy as np

B, L, D, H, NMODE, Dh = 8, 4096, 1024, 16, 3, 64
KT = D // 128              # k tiles (contraction)
MT = D // 128              # m tiles (output channels)
INVSQRT2 = 0.7071067811865476
# small first chunk so chunk-0 PE work matches the weight-DMA supply rate;
# small last chunk to shrink the kernel tail.
CHUNKS = [(0, 512), (512, 1024), (1536, 1024), (2560, 1024), (3584, 512)]

_CACHE = {}


def _build():
    import concourse.bacc as bacc
    import concourse.mybir as mybir
    import concourse.tile as tile

    f32, f32r = mybir.dt.float32, mybir.dt.float32r
    ident = mybir.ActivationFunctionType.Identity
    add_op, mult_op = mybir.AluOpType.add, mybir.AluOpType.mult
    C3 = INVSQRT2 ** 3

    nc = bacc.Bacc("TRN2", target_bir_lowering=False, debug=False)
    xt = nc.dram_tensor("xt", [D, L], f32, kind="ExternalInput")
    wqt = nc.dram_tensor("wqt", [D, D], f32r, kind="ExternalInput")
    wvt = nc.dram_tensor("wvt", [D, D], f32r, kind="ExternalInput")
    wot = nc.dram_tensor("wot", [D, D], f32r, kind="ExternalInput")
    scal = nc.dram_tensor("scal", [128, 48], f32, kind="ExternalInput")
    outt = nc.dram_tensor("outt", [D, L], f32, kind="ExternalOutput")

    with tile.TileContext(nc) as tc:
        with tc.tile_pool(name="wq", bufs=KT) as wq_pool, \
             tc.tile_pool(name="wv", bufs=KT) as wv_pool, \
             tc.tile_pool(name="wo", bufs=KT) as wo_pool, \
             tc.tile_pool(name="const", bufs=1) as const_pool, \
             tc.tile_pool(name="x", bufs=2) as x_pool, \
             tc.tile_pool(name="u", bufs=8) as u_pool, \
             tc.tile_pool(name="ta", bufs=2) as ta_pool, \
             tc.tile_pool(name="cq", bufs=2) as cq_pool, \
             tc.tile_pool(name="prod", bufs=2) as prod_pool, \
             tc.tile_pool(name="ti", bufs=2) as ti_pool, \
             tc.tile_pool(name="y", bufs=8) as y_pool, \
             tc.tile_pool(name="o", bufs=2) as o_pool, \
             tc.tile_pool(name="psum", bufs=8, space="PSUM") as psum_pool:

            scal_sb = const_pool.tile([128, 48], f32)
            nc.sync.dma_start(scal_sb[:], scal.ap())

            def sap(j, m):          # per-partition scalar column
                return scal_sb[:, j * 8 + m: j * 8 + m + 1]

            def dwt_tile(k, csl, S):
                """Load x^T tile and produce its unscaled Haar-3 bands."""
                xt_t = x_pool.tile([128, S], f32, tag="x")
                nc.sync.dma_start(xt_t[:], xt.ap()[k * 128:(k + 1) * 128, csl])
                ut = u_pool.tile([128, S], f32r, tag="u")
                b0, b1, b2 = S // 8, S // 4, S // 2
                e, o = xt_t[:, 0:S:2], xt_t[:, 1:S:2]
                nc.vector.tensor_sub(ut[:, b2:S], e, o)              # D1
                t1 = ta_pool.tile([128, b2], f32, tag="t1")
                nc.vector.tensor_add(t1[:], e, o)                    # A1
                e2, o2 = t1[:, 0:b2:2], t1[:, 1:b2:2]
                nc.vector.tensor_sub(ut[:, b1:b2], e2, o2)           # D2
                t2 = ta_pool.tile([128, b1], f32, tag="t2")
                nc.vector.tensor_add(t2[:], e2, o2)                  # A2
                e3, o3 = t2[:, 0:b1:2], t2[:, 1:b1:2]
                nc.vector.tensor_sub(ut[:, b0:b1], e3, o3)           # D3
                nc.vector.tensor_add(ut[:, 0:b0], e3, o3)            # A3
                return ut

            # startup: x chunk-0 first (feeds DWT), then wq, wv; wo last.
            S0 = CHUNKS[0][1]
            u_first = [dwt_tile(k, slice(0, S0), S0) for k in range(KT)]
            wq_t, wv_t, wo_t = [], [], []
            for k in range(KT):
                sl = slice(k * 128, (k + 1) * 128)
                t = wq_pool.tile([128, D], f32r, tag="wq")
                nc.sync.dma_start(t[:], wqt.ap()[sl, :])
                wq_t.append(t)
            for k in range(KT):
                sl = slice(k * 128, (k + 1) * 128)
                t = wv_pool.tile([128, D], f32r, tag="wv")
                nc.sync.dma_start(t[:], wvt.ap()[sl, :])
                wv_t.append(t)
            for k in range(KT):
                sl = slice(k * 128, (k + 1) * 128)
                t = wo_pool.tile([128, D], f32r, tag="wo")
                nc.sync.dma_start(t[:], wot.ap()[sl, :])
                wo_t.append(t)

            def wqap(k, m):
                return wq_t[k][:, m * 128:(m + 1) * 128]

            def wvap(k, m):
                return wv_t[k][:, m * 128:(m + 1) * 128]

            for ci, (l0, S) in enumerate(CHUNKS):
                csl = slice(l0, l0 + S)
                nh = S // 512            # matmul n-halves
                b0, b1, b2 = S // 8, S // 4, S // 2   # band boundaries
                # ---- stage A: unscaled Haar-3 DWT of x^T chunk ----
                if ci == 0:
                    u_c = u_first
                else:
                    u_c = [dwt_tile(k, csl, S) for k in range(KT)]

                # ---- stages B (project+weight+product) and C (iDWT) ----
                y_c = []
                for m in range(MT):
                    prod = prod_pool.tile([128, S], f32, tag="prod")
                    psq = []
                    for _i in range(nh):
                        pst = psum_pool.tile([128, 512], f32, tag="ps", name=f"psq{_i}")
                        psq.append(pst)
                    for k in range(KT):
                        for h in range(nh):
                            nc.tensor.matmul(psq[h][:], wqap(k, m),
                                             u_c[k][:, h * 512:(h + 1) * 512],
                                             start=(k == 0), stop=(k == KT - 1))
                    # band weights/scales fold into the PSUM->SBUF copy (ACT),
                    # emitted right after the q-chain so psq banks recycle.
                    cq = cq_pool.tile([128, S], f32, tag="cq")

                    def seg(lo, hi):
                        h, off = lo // 512, lo % 512
                        return cq[:, lo:hi], psq[h][:, off:off + (hi - lo)]
                    o_, i_ = seg(0, b0)
                    nc.scalar.activation(o_, i_, ident, bias=sap(3, m), scale=sap(0, m))
                    o_, i_ = seg(b0, b1)
                    nc.scalar.mul(o_, i_, sap(1, m))
                    o_, i_ = seg(b1, b2)
                    nc.scalar.mul(o_, i_, sap(2, m))
                    for lo in range(b2, S, 512):     # D1 region
                        o_, i_ = seg(lo, min(lo + 512, ((lo // 512) + 1) * 512))
                        nc.scalar.mul(o_, i_, C3)
                    psv = []
                    for _i in range(nh):
                        pst = psum_pool.tile([128, 512], f32, tag="ps", name=f"psv{_i}")
                        psv.append(pst)
                    for k in range(KT):
                        for h in range(nh):
                            nc.tensor.matmul(psv[h][:], wvap(k, m),
                                             u_c[k][:, h * 512:(h + 1) * 512],
                                             start=(k == 0), stop=(k == KT - 1))
                    # products: prod = cq * (psv [+ bias on A3])
                    nc.vector.scalar_tensor_tensor(
                        prod[:, 0:b0], psv[0][:, 0:b0], sap(4, m), cq[:, 0:b0],
                        op0=add_op, op1=mult_op)
                    for h in range(nh):
                        lo = b0 if h == 0 else h * 512
                        nc.vector.tensor_mul(prod[:, lo:(h + 1) * 512],
                                             cq[:, lo:(h + 1) * 512],
                                             psv[h][:, lo % 512:512])
                    # iDWT (unscaled butterflies, interleaved writes)
                    it2 = ti_pool.tile([128, b1], f32, tag="it2")
                    nc.vector.tensor_add(it2[:, 0:b1:2], prod[:, 0:b0], prod[:, b0:b1])
                    nc.vector.tensor_sub(it2[:, 1:b1:2], prod[:, 0:b0], prod[:, b0:b1])
                    it1 = ti_pool.tile([128, b2], f32, tag="it1")
                    nc.vector.tensor_add(it1[:, 0:b2:2], it2[:], prod[:, b1:b2])
                    nc.vector.tensor_sub(it1[:, 1:b2:2], it2[:], prod[:, b1:b2])
                    yt = y_pool.tile([128, S], f32r, tag="y")
                    nc.vector.tensor_add(yt[:, 0:S:2], it1[:], prod[:, b2:S])
                    nc.vector.tensor_sub(yt[:, 1:S:2], it1[:], prod[:, b2:S])
                    y_c.append(yt)

                # ---- stage D: output projection ----
                for m in range(MT):
                    msl = slice(m * 128, (m + 1) * 128)
                    pso = []
                    for _i in range(nh):
                        pst = psum_pool.tile([128, 512], f32, tag="ps", name=f"pso{_i}")
                        pso.append(pst)
                    for k in range(KT):
                        for h in range(nh):
                            nc.tensor.matmul(pso[h][:], wo_t[k][:, msl],
                                             y_c[k][:, h * 512:(h + 1) * 512],
                                             start=(k == 0), stop=(k == KT - 1))
                    ot = o_pool.tile([128, S], f32, tag="o")
                    for h in range(nh):
                        nc.scalar.copy(ot[:, h * 512:(h + 1) * 512], pso[h][:])
                    nc.sync.dma_start(outt.ap()[msl, csl], ot[:])
    nc.compile()
    return nc


def _prep(W_qkv, b_qkv, W_out, weight_q, weight_v):
    c = INVSQRT2
    WqT = np.ascontiguousarray(W_qkv[0:D, :].T, dtype=np.float32)
    WvT = np.ascontiguousarray(W_qkv[2 * D:3 * D, :].T, dtype=np.float32)
    WoT = np.ascontiguousarray(W_out.T, dtype=np.float32)
    wb = np.empty((3, D), np.float32)
    for j in range(3):
        wb[j] = (np.asarray(weight_q)[:, j, :] * np.asarray(weight_v)[:, j, :]).reshape(D)
    wb0 = wb[0] * c ** 9
    wb1 = wb[1] * c ** 9
    wb2 = wb[2] * c ** 6
    bq = np.asarray(b_qkv)[0:D].astype(np.float32)
    bv = np.asarray(b_qkv)[2 * D:3 * D].astype(np.float32)
    bqw8 = wb0 * 8.0 * bq
    bv8 = 8.0 * bv
    scal = np.zeros((128, 48), np.float32)
    for j, vec in enumerate((wb0, wb1, wb2, bqw8, bv8)):
        scal[:, j * 8:(j + 1) * 8] = vec.reshape(8, 128).T
    return WqT, WvT, WoT, scal


def kernel(query, W_qkv, b_qkv, W_out, b_out, weight_q, weight_v, _trace=False):
    from concourse.bass_utils import run_bass_kernel_spmd

    if "nc" not in _CACHE:
        _CACHE["nc"] = _build()
    nc = _CACHE["nc"]

    WqT, WvT, WoT, scal = _prep(W_qkv, b_qkv, W_out, weight_q, weight_v)
    query = np.asarray(query, dtype=np.float32)
    in_maps = []
    for b in range(B):
        in_maps.append({
            "xt": np.ascontiguousarray(query[b].T),
            "wqt": WqT, "wvt": WvT, "wot": WoT, "scal": scal,
        })
    res = run_bass_kernel_spmd(nc, in_maps, list(range(B)), trace=_trace)
    out = np.empty((B, L, D), np.float32)
    for b in range(B):
        out[b] = res.results[b]["outt"].T
    out += np.asarray(b_out, dtype=np.float32)[None, None, :]
    if _trace:
        _CACHE["last_results"] = res
    return out


# revision 16
# speedup vs baseline: 1.0391x; 1.0170x over previous
"""DWT-attention Trainium2 kernel.

Math (per batch b, all on device):
  out = ( iDWT3( W ⊙ (DWT3(x)Wq^T) ⊙ (DWT3(x)Wv^T) ) ) Wout^T + b_out
using that the Haar DWT along L commutes with channel projections, so the
DWT is applied ONCE to x (not to q and v separately), and all 1/sqrt(2)
band scales + the per-(head,channel) band weights are folded into
per-channel scalars applied on the PSUM->SBUF copy. b_out is added on the
host (it is additive at the very end).

Layout: everything transposed — x^T [D, L] so L is the free dim (DWT is
free-dim strided adds on DVE) and channels are partitions (band weights
become per-partition scalars). Sharding: batch B=8, one batch per core.
Matmuls run in float32r (fast-FP32 PE mode, ~1e-4 rel err).
"""
import sys
sys.path.insert(0, "/opt/trn_rl_repo")
import numpy as np

B, L, D, H, NMODE, Dh = 8, 4096, 1024, 16, 3, 64
KT = D // 128              # k tiles (contraction)
MT = D // 128              # m tiles (output channels)
INVSQRT2 = 0.7071067811865476
# small first chunk so chunk-0 PE work matches the weight-DMA supply rate;
# small last chunk to shrink the kernel tail.
CHUNKS = [(0, 512), (512, 1024), (1536, 1024), (2560, 1024), (3584, 512)]

_CACHE = {}


def _build():
    import concourse.bacc as bacc
    import concourse.mybir as mybir
    import concourse.tile as tile

    f32, f32r = mybir.dt.float32, mybir.dt.float32r
    ident = mybir.ActivationFunctionType.Identity
    add_op, mult_op = mybir.AluOpType.add, mybir.AluOpType.mult
    C3 = INVSQRT2 ** 3

    nc = bacc.Bacc("TRN2", target_bir_lowering=False, debug=False)
    xt = nc.dram_tensor("xt", [D, L], f32, kind="ExternalInput")
    wqt = nc.dram_tensor("wqt", [D, D], f32r, kind="ExternalInput")
    wvt = nc.dram_tensor("wvt", [D, D], f32r, kind="ExternalInput")
    wot = nc.dram_tensor("wot", [D, D], f32r, kind="ExternalInput")
    scal = nc.dram_tensor("scal", [128, 48], f32, kind="ExternalInput")
    outt = nc.dram_tensor("outt", [D, L], f32, kind="ExternalOutput")

    with tile.TileContext(nc) as tc:
        with tc.tile_pool(name="wq", bufs=KT) as wq_pool, \
             tc.tile_pool(name="wv", bufs=KT) as wv_pool, \
             tc.tile_pool(name="wo", bufs=KT) as wo_pool, \
             tc.tile_pool(name="const", bufs=1) as const_pool, \
             tc.tile_pool(name="x", bufs=2) as x_pool, \
             tc.tile_pool(name="u", bufs=8) as u_pool, \
             tc.tile_pool(name="ta", bufs=2) as ta_pool, \
             tc.tile_pool(name="cq", bufs=3) as cq_pool, \
             tc.tile_pool(name="prod", bufs=2) as prod_pool, \
             tc.tile_pool(name="ti", bufs=1) as ti_pool, \
             tc.tile_pool(name="y", bufs=8) as y_pool, \
             tc.tile_pool(name="o", bufs=2) as o_pool, \
             tc.tile_pool(name="psum", bufs=8, space="PSUM") as psum_pool:

            scal_sb = const_pool.tile([128, 48], f32)
            nc.sync.dma_start(scal_sb[:], scal.ap())

            def sap(j, m):          # per-partition scalar column
                return scal_sb[:, j * 8 + m: j * 8 + m + 1]

            def dwt_tile(k, csl, S):
                """Load x^T tile and produce its unscaled Haar-3 bands."""
                xt_t = x_pool.tile([128, S], f32, tag="x")
                nc.sync.dma_start(xt_t[:], xt.ap()[k * 128:(k + 1) * 128, csl])
                ut = u_pool.tile([128, S], f32r, tag="u")
                b0, b1, b2 = S // 8, S // 4, S // 2
                e, o = xt_t[:, 0:S:2], xt_t[:, 1:S:2]
                nc.vector.tensor_sub(ut[:, b2:S], e, o)              # D1
                t1 = ta_pool.tile([128, b2], f32, tag="t1")
                nc.vector.tensor_add(t1[:], e, o)                    # A1
                e2, o2 = t1[:, 0:b2:2], t1[:, 1:b2:2]
                nc.vector.tensor_sub(ut[:, b1:b2], e2, o2)           # D2
                t2 = ta_pool.tile([128, b1], f32, tag="t2")
                nc.vector.tensor_add(t2[:], e2, o2)                  # A2
                e3, o3 = t2[:, 0:b1:2], t2[:, 1:b1:2]
                nc.vector.tensor_sub(ut[:, b0:b1], e3, o3)           # D3
                nc.vector.tensor_add(ut[:, 0:b0], e3, o3)            # A3
                return ut

            # startup: x chunk-0 first (feeds DWT), then wq, wv; wo last.
            S0 = CHUNKS[0][1]
            u_first = [dwt_tile(k, slice(0, S0), S0) for k in range(KT)]
            wq_t, wv_t, wo_t = [], [], []
            for k in range(KT):
                sl = slice(k * 128, (k + 1) * 128)
                t = wq_pool.tile([128, D], f32r, tag="wq")
                nc.sync.dma_start(t[:], wqt.ap()[sl, :])
                wq_t.append(t)
            for k in range(KT):
                sl = slice(k * 128, (k + 1) * 128)
                t = wv_pool.tile([128, D], f32r, tag="wv")
                nc.sync.dma_start(t[:], wvt.ap()[sl, :])
                wv_t.append(t)
            for k in range(KT):
                sl = slice(k * 128, (k + 1) * 128)
                t = wo_pool.tile([128, D], f32r, tag="wo")
                nc.sync.dma_start(t[:], wot.ap()[sl, :])
                wo_t.append(t)

            def wqap(k, m):
                return wq_t[k][:, m * 128:(m + 1) * 128]

            def wvap(k, m):
                return wv_t[k][:, m * 128:(m + 1) * 128]

            for ci, (l0, S) in enumerate(CHUNKS):
                csl = slice(l0, l0 + S)
                nh = S // 512            # matmul n-halves
                b0, b1, b2 = S // 8, S // 4, S // 2   # band boundaries
                # ---- stage A: unscaled Haar-3 DWT of x^T chunk ----
                if ci == 0:
                    u_c = u_first
                else:
                    u_c = [dwt_tile(k, csl, S) for k in range(KT)]

                # ---- stages B (project+weight+product) and C (iDWT) ----
                def emit_q(m):
                    """q-chain + weighted PSUM->SBUF copy; returns cq tile."""
                    psq = []
                    for _i in range(nh):
                        pst = psum_pool.tile([128, 512], f32, tag="ps", name=f"psq{_i}")
                        psq.append(pst)
                    for k in range(KT):
                        for h in range(nh):
                            nc.tensor.matmul(psq[h][:], wqap(k, m),
                                             u_c[k][:, h * 512:(h + 1) * 512],
                                             start=(k == 0), stop=(k == KT - 1))
                    cq = cq_pool.tile([128, S], f32, tag="cq")

                    def seg(lo, hi):
                        h, off = lo // 512, lo % 512
                        return cq[:, lo:hi], psq[h][:, off:off + (hi - lo)]
                    o_, i_ = seg(0, b0)
                    nc.scalar.activation(o_, i_, ident, bias=sap(3, m), scale=sap(0, m))
                    o_, i_ = seg(b0, b1)
                    nc.scalar.mul(o_, i_, sap(1, m))
                    o_, i_ = seg(b1, b2)
                    nc.scalar.mul(o_, i_, sap(2, m))
                    for lo in range(b2, S, 512):     # D1 region
                        o_, i_ = seg(lo, min(lo + 512, ((lo // 512) + 1) * 512))
                        nc.scalar.mul(o_, i_, C3)
                    return cq

                def emit_v(m, cq):
                    """v-chain + products + iDWT; returns y tile."""
                    prod = prod_pool.tile([128, S], f32, tag="prod")
                    psv = []
                    for _i in range(nh):
                        pst = psum_pool.tile([128, 512], f32, tag="ps", name=f"psv{_i}")
                        psv.append(pst)
                    for k in range(KT):
                        for h in range(nh):
                            nc.tensor.matmul(psv[h][:], wvap(k, m),
                                             u_c[k][:, h * 512:(h + 1) * 512],
                                             start=(k == 0), stop=(k == KT - 1))
                    nc.vector.scalar_tensor_tensor(
                        prod[:, 0:b0], psv[0][:, 0:b0], sap(4, m), cq[:, 0:b0],
                        op0=add_op, op1=mult_op)
                    for h in range(nh):
                        lo = b0 if h == 0 else h * 512
                        nc.vector.tensor_mul(prod[:, lo:(h + 1) * 512],
                                             cq[:, lo:(h + 1) * 512],
                                             psv[h][:, lo % 512:512])
                    it2 = ti_pool.tile([128, b1], f32, tag="it2")
                    nc.vector.tensor_add(it2[:, 0:b1:2], prod[:, 0:b0], prod[:, b0:b1])
                    nc.vector.tensor_sub(it2[:, 1:b1:2], prod[:, 0:b0], prod[:, b0:b1])
                    it1 = ti_pool.tile([128, b2], f32, tag="it1")
                    nc.vector.tensor_add(it1[:, 0:b2:2], it2[:], prod[:, b1:b2])
                    nc.vector.tensor_sub(it1[:, 1:b2:2], it2[:], prod[:, b1:b2])
                    yt = y_pool.tile([128, S], f32r, tag="y")
                    nc.vector.tensor_add(yt[:, 0:S:2], it1[:], prod[:, b2:S])
                    nc.vector.tensor_sub(yt[:, 1:S:2], it1[:], prod[:, b2:S])
                    return yt

                # chunk 0: v-side lags q-side so the early PE stream only
                # needs wq (wv still streaming in).
                LAG = 2 if ci == 0 else 0
                y_c, cq_q = [], {}
                for mi in range(MT + LAG):
                    if mi < MT:
                        cq_q[mi] = emit_q(mi)
                    if mi >= LAG:
                        y_c.append(emit_v(mi - LAG, cq_q.pop(mi - LAG)))

                # ---- stage D: output projection ----
                for m in range(MT):
                    msl = slice(m * 128, (m + 1) * 128)
                    pso = []
                    for _i in range(nh):
                        pst = psum_pool.tile([128, 512], f32, tag="ps", name=f"pso{_i}")
                        pso.append(pst)
                    for k in range(KT):
                        for h in range(nh):
                            nc.tensor.matmul(pso[h][:], wo_t[k][:, msl],
                                             y_c[k][:, h * 512:(h + 1) * 512],
                                             start=(k == 0), stop=(k == KT - 1))
                    ot = o_pool.tile([128, S], f32, tag="o")
                    for h in range(nh):
                        nc.scalar.copy(ot[:, h * 512:(h + 1) * 512], pso[h][:])
                    nc.sync.dma_start(outt.ap()[msl, csl], ot[:])
    nc.compile()
    return nc


def _prep(W_qkv, b_qkv, W_out, weight_q, weight_v):
    c = INVSQRT2
    WqT = np.ascontiguousarray(W_qkv[0:D, :].T, dtype=np.float32)
    WvT = np.ascontiguousarray(W_qkv[2 * D:3 * D, :].T, dtype=np.float32)
    WoT = np.ascontiguousarray(W_out.T, dtype=np.float32)
    wb = np.empty((3, D), np.float32)
    for j in range(3):
        wb[j] = (np.asarray(weight_q)[:, j, :] * np.asarray(weight_v)[:, j, :]).reshape(D)
    wb0 = wb[0] * c ** 9
    wb1 = wb[1] * c ** 9
    wb2 = wb[2] * c ** 6
    bq = np.asarray(b_qkv)[0:D].astype(np.float32)
    bv = np.asarray(b_qkv)[2 * D:3 * D].astype(np.float32)
    bqw8 = wb0 * 8.0 * bq
    bv8 = 8.0 * bv
    scal = np.zeros((128, 48), np.float32)
    for j, vec in enumerate((wb0, wb1, wb2, bqw8, bv8)):
        scal[:, j * 8:(j + 1) * 8] = vec.reshape(8, 128).T
    return WqT, WvT, WoT, scal


def kernel(query, W_qkv, b_qkv, W_out, b_out, weight_q, weight_v, _trace=False):
    from concourse.bass_utils import run_bass_kernel_spmd

    if "nc" not in _CACHE:
        _CACHE["nc"] = _build()
    nc = _CACHE["nc"]

    WqT, WvT, WoT, scal = _prep(W_qkv, b_qkv, W_out, weight_q, weight_v)
    query = np.asarray(query, dtype=np.float32)
    in_maps = []
    for b in range(B):
        in_maps.append({
            "xt": np.ascontiguousarray(query[b].T),
            "wqt": WqT, "wvt": WvT, "wot": WoT, "scal": scal,
        })
    res = run_bass_kernel_spmd(nc, in_maps, list(range(B)), trace=_trace)
    out = np.empty((B, L, D), np.float32)
    for b in range(B):
        out[b] = res.results[b]["outt"].T
    out += np.asarray(b_out, dtype=np.float32)[None, None, :]
    if _trace:
        _CACHE["last_results"] = res
    return out


# revision 18
# speedup vs baseline: 1.0403x; 1.0011x over previous
"""DWT-attention Trainium2 kernel.

Math (per batch b, all on device):
  out = ( iDWT3( W ⊙ (DWT3(x)Wq^T) ⊙ (DWT3(x)Wv^T) ) ) Wout^T + b_out
using that the Haar DWT along L commutes with channel projections, so the
DWT is applied ONCE to x (not to q and v separately), and all 1/sqrt(2)
band scales + the per-(head,channel) band weights are folded into
per-channel scalars applied on the PSUM->SBUF copy. b_out is added on the
host (it is additive at the very end).

Layout: everything transposed — x^T [D, L] so L is the free dim (DWT is
free-dim strided adds on DVE) and channels are partitions (band weights
become per-partition scalars). Sharding: batch B=8, one batch per core.
Matmuls run in float32r (fast-FP32 PE mode, ~1e-4 rel err).
"""
import sys
sys.path.insert(0, "/opt/trn_rl_repo")
import numpy as np

B, L, D, H, NMODE, Dh = 8, 4096, 1024, 16, 3, 64
KT = D // 128              # k tiles (contraction)
MT = D // 128              # m tiles (output channels)
INVSQRT2 = 0.7071067811865476
# small first chunk so chunk-0 PE work matches the weight-DMA supply rate;
# small last chunk to shrink the kernel tail.
CHUNKS = [(0, 512), (512, 1024), (1536, 1024), (2560, 1024), (3584, 512)]

_CACHE = {}


def _build():
    import concourse.bacc as bacc
    import concourse.mybir as mybir
    import concourse.tile as tile

    f32, f32r = mybir.dt.float32, mybir.dt.float32r
    ident = mybir.ActivationFunctionType.Identity
    add_op, mult_op = mybir.AluOpType.add, mybir.AluOpType.mult
    C3 = INVSQRT2 ** 3

    nc = bacc.Bacc("TRN2", target_bir_lowering=False, debug=False)
    xt = nc.dram_tensor("xt", [D, L], f32, kind="ExternalInput")
    wqt = nc.dram_tensor("wqt", [D, D], f32r, kind="ExternalInput")
    wvt = nc.dram_tensor("wvt", [D, D], f32r, kind="ExternalInput")
    wot = nc.dram_tensor("wot", [D, D], f32r, kind="ExternalInput")
    scal = nc.dram_tensor("scal", [128, 48], f32, kind="ExternalInput")
    outt = nc.dram_tensor("outt", [D, L], f32, kind="ExternalOutput")

    with tile.TileContext(nc) as tc:
        with tc.tile_pool(name="wq", bufs=KT) as wq_pool, \
             tc.tile_pool(name="wv", bufs=KT) as wv_pool, \
             tc.tile_pool(name="wo", bufs=KT) as wo_pool, \
             tc.tile_pool(name="const", bufs=1) as const_pool, \
             tc.tile_pool(name="x", bufs=2) as x_pool, \
             tc.tile_pool(name="u", bufs=8) as u_pool, \
             tc.tile_pool(name="ta", bufs=1) as ta_pool, \
             tc.tile_pool(name="cq", bufs=4) as cq_pool, \
             tc.tile_pool(name="prod", bufs=2) as prod_pool, \
             tc.tile_pool(name="ti", bufs=1) as ti_pool, \
             tc.tile_pool(name="y", bufs=8) as y_pool, \
             tc.tile_pool(name="o", bufs=2) as o_pool, \
             tc.tile_pool(name="psum", bufs=8, space="PSUM") as psum_pool:

            scal_sb = const_pool.tile([128, 48], f32)
            nc.sync.dma_start(scal_sb[:], scal.ap())

            def sap(j, m):          # per-partition scalar column
                return scal_sb[:, j * 8 + m: j * 8 + m + 1]

            def dwt_tile(k, csl, S):
                """Load x^T tile and produce its unscaled Haar-3 bands."""
                xt_t = x_pool.tile([128, S], f32, tag="x")
                nc.sync.dma_start(xt_t[:], xt.ap()[k * 128:(k + 1) * 128, csl])
                ut = u_pool.tile([128, S], f32r, tag="u")
                b0, b1, b2 = S // 8, S // 4, S // 2
                e, o = xt_t[:, 0:S:2], xt_t[:, 1:S:2]
                nc.vector.tensor_sub(ut[:, b2:S], e, o)              # D1
                t1 = ta_pool.tile([128, b2], f32, tag="t1")
                nc.vector.tensor_add(t1[:], e, o)                    # A1
                e2, o2 = t1[:, 0:b2:2], t1[:, 1:b2:2]
                nc.vector.tensor_sub(ut[:, b1:b2], e2, o2)           # D2
                t2 = ta_pool.tile([128, b1], f32, tag="t2")
                nc.vector.tensor_add(t2[:], e2, o2)                  # A2
                e3, o3 = t2[:, 0:b1:2], t2[:, 1:b1:2]
                nc.vector.tensor_sub(ut[:, b0:b1], e3, o3)           # D3
                nc.vector.tensor_add(ut[:, 0:b0], e3, o3)            # A3
                return ut

            # startup: x chunk-0 first (feeds DWT), then wq, wv; wo last.
            S0 = CHUNKS[0][1]
            u_first = [dwt_tile(k, slice(0, S0), S0) for k in range(KT)]
            wq_t, wv_t, wo_t = [], [], []
            for k in range(KT):
                sl = slice(k * 128, (k + 1) * 128)
                t = wq_pool.tile([128, D], f32r, tag="wq")
                nc.sync.dma_start(t[:], wqt.ap()[sl, :])
                wq_t.append(t)
            for k in range(KT):
                sl = slice(k * 128, (k + 1) * 128)
                t = wv_pool.tile([128, D], f32r, tag="wv")
                nc.sync.dma_start(t[:], wvt.ap()[sl, :])
                wv_t.append(t)
            for k in range(KT):
                sl = slice(k * 128, (k + 1) * 128)
                t = wo_pool.tile([128, D], f32r, tag="wo")
                nc.sync.dma_start(t[:], wot.ap()[sl, :])
                wo_t.append(t)

            def wqap(k, m):
                return wq_t[k][:, m * 128:(m + 1) * 128]

            def wvap(k, m):
                return wv_t[k][:, m * 128:(m + 1) * 128]

            for ci, (l0, S) in enumerate(CHUNKS):
                csl = slice(l0, l0 + S)
                nh = S // 512            # matmul n-halves
                b0, b1, b2 = S // 8, S // 4, S // 2   # band boundaries
                # ---- stage A: unscaled Haar-3 DWT of x^T chunk ----
                if ci == 0:
                    u_c = u_first
                else:
                    u_c = [dwt_tile(k, csl, S) for k in range(KT)]

                # ---- stages B (project+weight+product) and C (iDWT) ----
                def emit_q(m):
                    """q-chain + weighted PSUM->SBUF copy; returns cq tile."""
                    psq = []
                    for _i in range(nh):
                        pst = psum_pool.tile([128, 512], f32, tag="ps", name=f"psq{_i}")
                        psq.append(pst)
                    for k in range(KT):
                        for h in range(nh):
                            nc.tensor.matmul(psq[h][:], wqap(k, m),
                                             u_c[k][:, h * 512:(h + 1) * 512],
                                             start=(k == 0), stop=(k == KT - 1))
                    cq = cq_pool.tile([128, S], f32, tag="cq")

                    def seg(lo, hi):
                        h, off = lo // 512, lo % 512
                        return cq[:, lo:hi], psq[h][:, off:off + (hi - lo)]
                    o_, i_ = seg(0, b0)
                    nc.scalar.activation(o_, i_, ident, bias=sap(3, m), scale=sap(0, m))
                    o_, i_ = seg(b0, b1)
                    nc.scalar.mul(o_, i_, sap(1, m))
                    o_, i_ = seg(b1, b2)
                    nc.scalar.mul(o_, i_, sap(2, m))
                    for lo in range(b2, S, 512):     # D1 region
                        o_, i_ = seg(lo, min(lo + 512, ((lo // 512) + 1) * 512))
                        nc.scalar.mul(o_, i_, C3)
                    return cq

                def emit_v(m, cq):
                    """v-chain + products + iDWT; returns y tile."""
                    prod = prod_pool.tile([128, S], f32, tag="prod")
                    psv = []
                    for _i in range(nh):
                        pst = psum_pool.tile([128, 512], f32, tag="ps", name=f"psv{_i}")
                        psv.append(pst)
                    for k in range(KT):
                        for h in range(nh):
                            nc.tensor.matmul(psv[h][:], wvap(k, m),
                                             u_c[k][:, h * 512:(h + 1) * 512],
                                             start=(k == 0), stop=(k == KT - 1))
                    nc.vector.scalar_tensor_tensor(
                        prod[:, 0:b0], psv[0][:, 0:b0], sap(4, m), cq[:, 0:b0],
                        op0=add_op, op1=mult_op)
                    for h in range(nh):
                        lo = b0 if h == 0 else h * 512
                        nc.vector.tensor_mul(prod[:, lo:(h + 1) * 512],
                                             cq[:, lo:(h + 1) * 512],
                                             psv[h][:, lo % 512:512])
                    it2 = ti_pool.tile([128, b1], f32, tag="it2")
                    nc.vector.tensor_add(it2[:, 0:b1:2], prod[:, 0:b0], prod[:, b0:b1])
                    nc.vector.tensor_sub(it2[:, 1:b1:2], prod[:, 0:b0], prod[:, b0:b1])
                    it1 = ti_pool.tile([128, b2], f32, tag="it1")
                    nc.vector.tensor_add(it1[:, 0:b2:2], it2[:], prod[:, b1:b2])
                    nc.vector.tensor_sub(it1[:, 1:b2:2], it2[:], prod[:, b1:b2])
                    yt = y_pool.tile([128, S], f32r, tag="y")
                    nc.vector.tensor_add(yt[:, 0:S:2], it1[:], prod[:, b2:S])
                    nc.vector.tensor_sub(yt[:, 1:S:2], it1[:], prod[:, b2:S])
                    return yt

                # chunk 0: v-side lags q-side so the early PE stream only
                # needs wq (wv still streaming in).
                LAG = 3 if ci == 0 else 0
                y_c, cq_q = [], {}
                for mi in range(MT + LAG):
                    if mi < MT:
                        cq_q[mi] = emit_q(mi)
                    if mi >= LAG:
                        y_c.append(emit_v(mi - LAG, cq_q.pop(mi - LAG)))

                # ---- stage D: output projection ----
                for m in range(MT):
                    msl = slice(m * 128, (m + 1) * 128)
                    pso = []
                    for _i in range(nh):
                        pst = psum_pool.tile([128, 512], f32, tag="ps", name=f"pso{_i}")
                        pso.append(pst)
                    for k in range(KT):
                        for h in range(nh):
                            nc.tensor.matmul(pso[h][:], wo_t[k][:, msl],
                                             y_c[k][:, h * 512:(h + 1) * 512],
                                             start=(k == 0), stop=(k == KT - 1))
                    ot = o_pool.tile([128, S], f32, tag="o")
                    for h in range(nh):
                        nc.scalar.copy(ot[:, h * 512:(h + 1) * 512], pso[h][:])
                    nc.sync.dma_start(outt.ap()[msl, csl], ot[:])
    nc.compile()
    return nc


def _prep(W_qkv, b_qkv, W_out, weight_q, weight_v):
    c = INVSQRT2
    WqT = np.ascontiguousarray(W_qkv[0:D, :].T, dtype=np.float32)
    WvT = np.ascontiguousarray(W_qkv[2 * D:3 * D, :].T, dtype=np.float32)
    WoT = np.ascontiguousarray(W_out.T, dtype=np.float32)
    wb = np.empty((3, D), np.float32)
    for j in range(3):
        wb[j] = (np.asarray(weight_q)[:, j, :] * np.asarray(weight_v)[:, j, :]).reshape(D)
    wb0 = wb[0] * c ** 9
    wb1 = wb[1] * c ** 9
    wb2 = wb[2] * c ** 6
    bq = np.asarray(b_qkv)[0:D].astype(np.float32)
    bv = np.asarray(b_qkv)[2 * D:3 * D].astype(np.float32)
    bqw8 = wb0 * 8.0 * bq
    bv8 = 8.0 * bv
    scal = np.zeros((128, 48), np.float32)
    for j, vec in enumerate((wb0, wb1, wb2, bqw8, bv8)):
        scal[:, j * 8:(j + 1) * 8] = vec.reshape(8, 128).T
    return WqT, WvT, WoT, scal


def kernel(query, W_qkv, b_qkv, W_out, b_out, weight_q, weight_v, _trace=False):
    from concourse.bass_utils import run_bass_kernel_spmd

    if "nc" not in _CACHE:
        _CACHE["nc"] = _build()
    nc = _CACHE["nc"]

    WqT, WvT, WoT, scal = _prep(W_qkv, b_qkv, W_out, weight_q, weight_v)
    query = np.asarray(query, dtype=np.float32)
    in_maps = []
    for b in range(B):
        in_maps.append({
            "xt": np.ascontiguousarray(query[b].T),
            "wqt": WqT, "wvt": WvT, "wot": WoT, "scal": scal,
        })
    res = run_bass_kernel_spmd(nc, in_maps, list(range(B)), trace=_trace)
    out = np.empty((B, L, D), np.float32)
    for b in range(B):
        out[b] = res.results[b]["outt"].T
    out += np.asarray(b_out, dtype=np.float32)[None, None, :]
    if _trace:
        _CACHE["last_results"] = res
    return out
